# revision 1
# baseline (speedup 1.0000x reference)
"""Trainium2 Bass kernel for nn_LinearTextEmbedding.

out[n, c, x, y] = 1.0 if |bits[n, (512*x + y) % 1024]| > 0.5 else 0.0

Key structure: the flattened 512*512 map is the 1024-element thresholded
bit pattern tiled 256 times, and all 16 channels are identical.  So per
sample the kernel only has to materialize one 4 KiB pattern row per
partition and fan it out to DRAM; everything else is DMA-fabric-bound
store bandwidth (~427 GB/s combined across the two HWDGE rings when HBM
is quiet — the queue traces show the store phase runs gap-free at ring
rate, so the only improvable time is the load/compute ramp).

Structure (profiled min 184.6 us; store-drain floor is ~157 us at
fabric rate plus ~12 us ramp + tail):
  - 4 per-sample broadcast loads (alternating the two HWDGE rings) fill
    a single-copy [128, 4096] tile: every partition gets sample s's
    1024 bits at col s*1024.  Loading one copy instead of [pattern,
    pattern] halves the load traffic to 2 MiB; the x2 repeat moves into
    the store's dst access pattern (which must stay <= 3 dims, so the
    repeat rides the per-channel store, not a per-sample one).
  - per sample, 2 vector ops threshold its 1024-column subtile in place
    (x*x > 0.25  ==  |x| > 0.5 away from the representability boundary;
    abs_max fails the TRN2 TensorScalar ISA check.  test.py checks 0
    mismatches on the real inputs).  Per-sample subtiles let sample 0's
    stores launch while later loads are still settling.
  - per (sample, channel), one 1 MiB store (channel parity picks the
    ring): src re-reads the sample's 4 KiB partition row twice
    (stride-0 middle dim), dst lays the two copies at +0 and +1024 of
    the channel's partition chunk.

Sharding: pure data parallel, 32 samples -> 8 cores x 4 samples.

kernel() runs via run_bass_kernel_spmd on first call; repeat calls with
the same shapes reuse a cached jit executable (run_bass_kernel_spmd
rebuilds its jit wrapper every call, which would recompile HLO).
"""

import numpy as np

import concourse.bass as bass
import concourse.bacc as bacc
import concourse.mybir as mybir
import concourse.tile as tile
from concourse.bass_utils import run_bass_kernel_spmd

F32 = mybir.dt.float32

B = 32          # full batch
NBITS = 1024
NCORES = 8
BPC = B // NCORES   # samples per core
CH = 16
W = H = 512
MAP = W * H         # 262144 = 256 repeats of the 1024 pattern
SCOLS = BPC * NBITS  # 4096: one pattern copy per sample

_NC_CACHE = None
_JIT_CACHE = None


def _build():
    nc = bacc.Bacc(None, target_bir_lowering=False)
    bits = nc.dram_tensor("bits", [BPC, NBITS], F32, kind="ExternalInput")
    out = nc.dram_tensor("out", [BPC, CH, MAP], F32, kind="ExternalOutput")

    with tile.TileContext(nc) as tc:
        with tc.tile_pool(name="pool", bufs=1) as pool:
            rep = pool.tile([128, SCOLS], F32)
            ap = rep[:]
            for s in range(BPC):
                dst = bass.AP(ap.tensor, s * NBITS,
                              [[SCOLS, 128], [1, NBITS]])
                src = bass.AP(bits, s * NBITS, [[0, 128], [1, NBITS]])
                eng = nc.sync if s % 2 == 0 else nc.scalar
                eng.dma_start(dst, src)
            for s in range(BPC):
                # sub = (|sub| > 0.5) ? 1.0 : 0.0, via x*x > 0.25
                if s == 0:
                    # fast start: threshold s0 in two 512-col chunks and
                    # issue 512-col first stores so both rings begin
                    # draining ~1 us earlier (the first store is gated
                    # by load_s0 + this compute; everything later is
                    # ring-throughput-bound, not latency-bound)
                    for k in range(2):
                        sub = rep[:, k * 512:(k + 1) * 512]
                        nc.vector.tensor_mul(sub, sub, sub)
                        nc.vector.tensor_scalar(sub, sub, 0.25, None,
                                                op0=mybir.AluOpType.is_gt)
                        for c, eng in ((0, nc.sync), (1, nc.scalar)):
                            ssrc = bass.AP(ap.tensor, k * 512,
                                           [[SCOLS, 128], [0, 2], [1, 512]])
                            dst = bass.AP(out, c * MAP + k * 512,
                                          [[2 * NBITS, 128], [NBITS, 2],
                                           [1, 512]])
                            eng.dma_start(dst, ssrc)
                    chans = range(2, CH)
                else:
                    sub = rep[:, s * NBITS:(s + 1) * NBITS]
                    nc.vector.tensor_mul(sub, sub, sub)
                    nc.vector.tensor_scalar(sub, sub, 0.25, None,
                                            op0=mybir.AluOpType.is_gt)
                    chans = range(CH)
                for c in chans:
                    eng = nc.sync if c % 2 == 0 else nc.scalar
                    ssrc = bass.AP(ap.tensor, s * NBITS,
                                   [[SCOLS, 128], [0, 2], [1, NBITS]])
                    dst = bass.AP(out, (s * CH + c) * MAP,
                                  [[2 * NBITS, 128], [NBITS, 2], [1, NBITS]])
                    eng.dma_start(dst, ssrc)
    return nc


def _get_nc():
    global _NC_CACHE
    if _NC_CACHE is None:
        nc = _build()
        # run_bass_via_pjrt serializes nc.m as-is; Bacc defers register
        # allocation to finalize(), so finalize here or walrus sees
        # unallocated registers.
        nc.finalize()
        _NC_CACHE = nc
    return _NC_CACHE


def _run_cached(bits: np.ndarray, fetch: bool = True):
    """Repeat-call fast path: persistent jit executable + device-resident
    operand buffers.  No donation: the kernel writes every output byte,
    so result-buffer initialization is irrelevant.  fetch=False warms the
    executable (device run only) without pulling 512 MiB to the host."""
    global _JIT_CACHE
    import jax
    from jax.sharding import Mesh, PartitionSpec
    from jax.experimental.shard_map import shard_map
    import concourse.bass2jax as b2j

    nc = _get_nc()
    if _JIT_CACHE is None:
        partition_name = (
            nc.partition_id_tensor.name if nc.partition_id_tensor else None
        )
        in_names, out_names, out_avals, zero_outs = [], [], [], []
        for alloc in nc.m.functions[0].allocations:
            if not isinstance(alloc, b2j.mybir.MemoryLocationSet):
                continue
            name = alloc.memorylocations[0].name
            if alloc.kind == "ExternalInput":
                if name != partition_name:
                    in_names.append(name)
            elif alloc.kind == "ExternalOutput":
                shape = tuple(alloc.tensor_shape)
                dtype = b2j.mybir.dt.np(alloc.dtype)
                out_names.append(name)
                out_avals.append(jax.core.ShapedArray(shape, dtype))
                zero_outs.append(np.zeros(shape, dtype))
        n_params = len(in_names)
        all_in_names = in_names + out_names + (
            [partition_name] if partition_name else []
        )

        def _body(*args):
            operands = list(args)
            if partition_name is not None:
                operands.append(b2j.partition_id_tensor())
            return tuple(
                b2j._bass_exec_p.bind(
                    *operands,
                    out_avals=tuple(out_avals),
                    in_names=tuple(all_in_names),
                    out_names=tuple(out_names),
                    lowering_input_output_aliases=(),
                    sim_require_finite=True,
                    sim_require_nnan=True,
                    nc=nc,
                )
            )

        devices = jax.devices()[:NCORES]
        mesh = Mesh(np.asarray(devices), ("core",))
        nin = n_params + len(zero_outs)
        sharded = jax.jit(
            shard_map(_body, mesh=mesh,
                      in_specs=(PartitionSpec("core"),) * nin,
                      out_specs=(PartitionSpec("core"),) * len(out_names),
                      check_rep=False),
            keep_unused=True,
        )
        dev_zeros = [
            jax.device_put(np.zeros((NCORES * z.shape[0], *z.shape[1:]),
                                    z.dtype))
            for z in zero_outs
        ]
        _JIT_CACHE = (sharded, dev_zeros)

    sharded, dev_zeros = _JIT_CACHE
    out = sharded(np.ascontiguousarray(bits.astype(np.float32)), *dev_zeros)
    if not fetch:
        import jax
        jax.block_until_ready(out)
        return None
    return np.asarray(out[0]).reshape(B, CH, W, H)


def run_sharded(bits: np.ndarray, **spmd_kwargs):
    """Run on 8 cores; returns (full_output, BassKernelResults)."""
    nc = _get_nc()
    bits = np.ascontiguousarray(np.asarray(bits, dtype=np.float32))
    assert bits.shape == (B, NBITS), bits.shape
    in_maps = [
        {"bits": bits[k * BPC:(k + 1) * BPC]} for k in range(NCORES)
    ]
    res = run_bass_kernel_spmd(nc, in_maps, list(range(NCORES)), **spmd_kwargs)
    outs = [
        np.asarray(res.results[k]["out"]).reshape(BPC, CH, W, H)
        for k in range(NCORES)
    ]
    return np.concatenate(outs, axis=0), res


def kernel(bits: np.ndarray) -> np.ndarray:
    if _JIT_CACHE is not None:
        return _run_cached(bits)
    full, _ = run_sharded(bits)
    # warm the repeat-call path so a timing loop over kernel() measures
    # executable dispatch, not per-call jit reconstruction
    try:
        _run_cached(bits, fetch=False)
    except Exception:
        pass
    return full


if __name__ == "__main__":
    rng = np.random.default_rng(0)
    x = rng.standard_normal((B, NBITS)).astype(np.float32)
    y = kernel(x)
    i = np.arange(W * H)
    vals = (np.abs(x[:, i % NBITS]) > 0.5).astype(np.float32)
    exp = np.broadcast_to(vals[:, None, :], (B, CH, W * H)).reshape(
        B, CH, W, H)
    print("mismatches:", int((y != exp).sum()), "/", y.size)

